# revision 1
# baseline (speedup 1.0000x reference)
"""Trainium2 Bass kernel for nn_ExactDivergenceModel (retrieval_knn).

Math (per batch b):
  XX[i,j] = ||X[i]-X[j]||, YX[i,j] = ||X[i]-Y[j]||
  out[b]  = (1/N) sum_i ( log min_{j!=i} XX[i,j] - log min_j YX[i,j] )
which only needs per-row minima of the squared-distance matrices:
  d2_XX[i,j] = x2[j] - 2<X_i,X_j>  (+ x2[i] added on host)
  d2_YX[i,j] = y2[j] - 2<X_i,Y_j>  (+ x2[i] added on host)

Device strategy (1 batch per NeuronCore, 8 cores):
  - Augmented matmul, K = D+2 = 66: lhsT = [-2*X^T; 1; 1], rhs = [Y^T; y2_hi; y2_lo]
    so PSUM directly holds y2[j] - 2<X_i, Y_j>. fp32r matmuls (1 cyc/row).
  - Diagonal of XX masked by accumulating BIG*I via a second matmul
    (lhsT = rhs = sqrt(BIG)*I_128, start=False) - PE-only, no vector cost.
  - Row minima via VectorE tensor_reduce(min) over PSUM half-rows [128, 2048].
  - Host adds x2[i], applies eps clamp + log + mean in float64.
"""
import sys
sys.path.insert(0, '/opt/trn_rl_repo')

import numpy as np
import ml_dtypes

import concourse.bass as bass
import concourse.tile as tile
from concourse import bacc, mybir
from concourse.bass_utils import run_bass_kernel_spmd

B, N, D = 8, 4096, 64
P = 128                 # partitions / i-block size
NB = N // P             # 32 i-blocks
K = D + 2               # contraction with the two norm rows
HALF = 2048             # psum half-row width
EPS = 1e-12
SQRT_BIG = 32768.0      # BIG = 2^30 on the XX diagonal

_cache = {}


def _build(repeat=1):
    nc = bacc.Bacc(None, target_bir_lowering=False)
    f32 = mybir.dt.float32
    f32r = mybir.dt.float32r

    L_d = nc.dram_tensor("L", [K, N], f32, kind="ExternalInput")
    RX_d = nc.dram_tensor("RX", [K, N], f32, kind="ExternalInput")
    RY_d = nc.dram_tensor("RY", [K, N], f32, kind="ExternalInput")
    EYE_d = nc.dram_tensor("EYE", [P, P], f32, kind="ExternalInput")
    MX_d = nc.dram_tensor("MX", [P, 2 * NB], f32, kind="ExternalOutput")
    MY_d = nc.dram_tensor("MY", [P, 2 * NB], f32, kind="ExternalOutput")

    with tile.TileContext(nc) as tc:
        with tc.tile_pool(name="const", bufs=1) as const, \
             tc.tile_pool(name="psum", bufs=2, space="PSUM") as psum, \
             tc.tile_pool(name="outs", bufs=1) as outs:
            Lf = const.tile([K, N], f32)
            RXf = const.tile([K, N], f32)
            RYf = const.tile([K, N], f32)
            EYEf = const.tile([P, P], f32)
            nc.sync.dma_start(out=Lf, in_=L_d[:])
            nc.sync.dma_start(out=RXf, in_=RX_d[:])
            nc.sync.dma_start(out=RYf, in_=RY_d[:])
            nc.sync.dma_start(out=EYEf, in_=EYE_d[:])

            Lr = const.tile([K, N], f32r)
            RXr = const.tile([K, N], f32r)
            RYr = const.tile([K, N], f32r)
            EYEr = const.tile([P, P], f32r)
            nc.vector.tensor_copy(Lr, Lf)
            nc.vector.tensor_copy(RXr, RXf)
            nc.vector.tensor_copy(RYr, RYf)
            nc.vector.tensor_copy(EYEr, EYEf)

            mins_x = outs.tile([P, 2 * NB], f32)
            mins_y = outs.tile([P, 2 * NB], f32)

            for _ in range(repeat):
                for bi in range(NB):
                    lhs = Lr[:, bi * P:(bi + 1) * P]
                    for R, mins, is_xx in ((RXr, mins_x, True), (RYr, mins_y, False)):
                        for h in range(2):
                            p = psum.tile([P, HALF], f32, tag="p")
                            for c in range(4):
                                jc = h * 4 + c
                                col0 = jc * 512
                                diag_here = is_xx and (bi * P) // 512 == jc
                                nc.tensor.matmul(
                                    p[:, c * 512:(c + 1) * 512],
                                    lhs, R[:, col0:col0 + 512],
                                    start=True, stop=not diag_here)
                                if diag_here:
                                    off = c * 512 + (bi * P - col0)
                                    nc.tensor.matmul(
                                        p[:, off:off + P], EYEr[:], EYEr[:],
                                        start=False, stop=True,
                                        skip_group_check=True)
                            nc.vector.tensor_reduce(
                                out=mins[:, 2 * bi + h:2 * bi + h + 1],
                                in_=p[:], axis=mybir.AxisListType.X,
                                op=mybir.AluOpType.min)

            nc.sync.dma_start(out=MX_d[:], in_=mins_x)
            nc.sync.dma_start(out=MY_d[:], in_=mins_y)

    nc.finalize()
    return nc


def _get_nc(repeat=1):
    if repeat not in _cache:
        _cache[repeat] = _build(repeat)
    return _cache[repeat]


def _bf16_round(v):
    return v.astype(np.float32).astype(ml_dtypes.bfloat16).astype(np.float64)


def _prep_maps(X, Y):
    X = np.asarray(X, dtype=np.float32)
    Y = np.asarray(Y, dtype=np.float32)
    eye = (np.eye(P) * SQRT_BIG).astype(np.float32)
    in_maps = []
    x2_all = []
    for b in range(B):
        Xb = X[b].astype(np.float64)
        Yb = Y[b].astype(np.float64)
        x2 = (Xb * Xb).sum(1)
        y2 = (Yb * Yb).sum(1)
        ones = np.ones((1, N), dtype=np.float64)
        L = np.concatenate([-2.0 * Xb.T, ones, ones], 0).astype(np.float32)
        x2h = _bf16_round(x2)
        y2h = _bf16_round(y2)
        RX = np.concatenate([Xb.T, x2h[None], (x2 - x2h)[None]], 0).astype(np.float32)
        RY = np.concatenate([Yb.T, y2h[None], (y2 - y2h)[None]], 0).astype(np.float32)
        in_maps.append({"L": L, "RX": RX, "RY": RY, "EYE": eye})
        x2_all.append(x2)
    return in_maps, x2_all


def _postprocess(results, x2_all):
    out = np.zeros(B, dtype=np.float64)
    for b in range(B):
        mx = results[b]["MX"].astype(np.float64)  # [P, 2*NB]
        my = results[b]["MY"].astype(np.float64)
        # [p, bi, h] -> min over halves -> [p, bi] -> row i = bi*P + p
        d2x = mx.reshape(P, NB, 2).min(2).T.reshape(-1)
        d2y = my.reshape(P, NB, 2).min(2).T.reshape(-1)
        d2x = d2x + x2_all[b]
        d2y = d2y + x2_all[b]
        d2x = np.maximum(d2x, EPS)
        d2y = np.maximum(d2y, EPS)
        out[b] = 0.5 * np.mean(np.log(d2x) - np.log(d2y))
    return out.astype(np.float32)


def kernel(X, Y):
    nc = _get_nc(repeat=1)
    in_maps, x2_all = _prep_maps(X, Y)
    res = run_bass_kernel_spmd(nc, in_maps, core_ids=list(range(B)))
    return _postprocess(res.results, x2_all)


if __name__ == "__main__":
    rng = np.random.default_rng(0)
    X = rng.standard_normal((B, N, D)).astype(np.float32)
    Y = rng.standard_normal((B, N, D)).astype(np.float32)
    print(kernel(X, Y))


# revision 7
# speedup vs baseline: 2.2362x; 2.2362x over previous
"""Trainium2 Bass kernel for nn_ExactDivergenceModel (retrieval_knn).

Math (per batch b):
  XX[i,j] = ||X[i]-X[j]||, YX[i,j] = ||X[i]-Y[j]||
  out[b]  = (1/N) sum_i ( log min_{j!=i} XX[i,j] - log min_j YX[i,j] )
which only needs per-row minima of the squared-distance matrices:
  d2_XX[i,j] = x2[j] - 2<X_i,X_j>  (+ x2[i] added on host)
  d2_YX[i,j] = y2[j] - 2<X_i,Y_j>  (+ x2[i] added on host)

Device strategy (1 batch per NeuronCore, 8 cores):
  - Augmented matmul, K = D+2 = 66: lhsT = [-2*X^T; 1; 1], rhs = [Y^T; y2_hi; y2_lo]
    so PSUM directly holds y2[j] - 2<X_i, Y_j>. fp32r matmuls (1 cyc/row).
  - Diagonal of XX masked by accumulating BIG*I via a second matmul
    (lhsT = rhs = sqrt(BIG)*I_128, start=False) - PE-only, no vector cost.
  - Row minima via VectorE tensor_reduce(min) over PSUM half-rows [128, 2048].
  - Host adds x2[i], applies eps clamp + log + mean in float64.
"""
import sys, time
sys.path.insert(0, '/opt/trn_rl_repo')

import numpy as np
import ml_dtypes

import concourse.bass as bass
import concourse.tile as tile
from concourse import bacc, mybir
from concourse.bass_utils import run_bass_kernel_spmd

B, N, D = 8, 4096, 64
P = 128                 # partitions / i-block size
NB = N // P             # 32 i-blocks
K = D + 2               # contraction with the two norm rows
HALF = 2048             # psum half-row width
EPS = 1e-12
SQRT_BIG = 32768.0      # BIG = 2^30 on the XX diagonal

_cache = {}

MM_DTYPE = "float32r"   # "float32r" | "float16" | "bfloat16" | "bf16x2"
MM_W = 512              # matmul free-dim width (chunk)
RED_W = 4096            # reduce width (psum tile width)


def _build(repeat=1, mmdt_name=None, mm_w=None, red_w=None):
    mmdt_name = mmdt_name or MM_DTYPE
    mm_w = mm_w or MM_W
    red_w = red_w or RED_W
    assert red_w % mm_w == 0 and N % red_w == 0
    n_tiles = N // red_w            # psum tiles per (block, matrix)
    n_ch = red_w // mm_w            # matmuls per psum tile
    psum_bufs = 1 if red_w == 4096 else 2
    nc = bacc.Bacc(None, target_bir_lowering=False)
    f32 = mybir.dt.float32
    mmdt = None if mmdt_name == "bf16x2" else getattr(mybir.dt, mmdt_name)

    bf16x2 = mmdt_name == "bf16x2"
    if bf16x2:
        K1, K2 = D + 2, 2 * D
        L_d = nc.dram_tensor("L", [K1, N], f32, kind="ExternalInput")     # [-2Xhi^T; 1]
        RX_d = nc.dram_tensor("RX", [K1, N], f32, kind="ExternalInput")   # [Xhi^T; x2h]
        RY_d = nc.dram_tensor("RY", [K1, N], f32, kind="ExternalInput")
        L2_d = nc.dram_tensor("L2", [K2, N], f32, kind="ExternalInput")   # [-2Xhi^T; -2Xlo^T; 1]
        RX2_d = nc.dram_tensor("RX2", [K2, N], f32, kind="ExternalInput") # [Xlo^T; Xhi^T; x2lo]
        RY2_d = nc.dram_tensor("RY2", [K2, N], f32, kind="ExternalInput")
        mmdt = mybir.dt.bfloat16
    else:
        RX_d = nc.dram_tensor("RX", [K, N], f32, kind="ExternalInput")
        RY_d = nc.dram_tensor("RY", [K, N], f32, kind="ExternalInput")
    EYE_d = nc.dram_tensor("EYE", [P, P], f32, kind="ExternalInput")
    MX_d = nc.dram_tensor("MX", [P, n_tiles * NB], f32, kind="ExternalOutput")
    MY_d = nc.dram_tensor("MY", [P, n_tiles * NB], f32, kind="ExternalOutput")

    with tile.TileContext(nc) as tc:
        with tc.tile_pool(name="const", bufs=1) as const, \
             tc.tile_pool(name="psum", bufs=psum_bufs, space="PSUM") as psum, \
             tc.tile_pool(name="outs", bufs=1) as outs:
            KA = (D + 2) if bf16x2 else K
            Lf = const.tile([KA, N], f32)
            RXf = const.tile([KA, N], f32)
            RYf = const.tile([KA, N], f32)
            EYEf = const.tile([P, P], f32)
            if bf16x2:
                nc.sync.dma_start(out=Lf, in_=L_d[:])
            else:
                # L = [-2*X^T; 1; 1] derived from RX = [X^T; x2h; x2l]
                nc.vector.memset(Lf[D:D + 2, :], 1.0)
            nc.sync.dma_start(out=RXf, in_=RX_d[:])
            nc.sync.dma_start(out=RYf, in_=RY_d[:])
            nc.sync.dma_start(out=EYEf, in_=EYE_d[:])
            if not bf16x2:
                nc.vector.tensor_scalar_mul(Lf[0:D, :], RXf[0:D, :], -2.0)

            Lr = const.tile([KA, N], mmdt)
            RXr = const.tile([KA, N], mmdt)
            RYr = const.tile([KA, N], mmdt)
            EYEr = const.tile([P, P], mmdt)
            nc.vector.tensor_copy(Lr, Lf)
            nc.vector.tensor_copy(RXr, RXf)
            nc.vector.tensor_copy(RYr, RYf)
            nc.vector.tensor_copy(EYEr, EYEf)
            if bf16x2:
                L2f = const.tile([K2, N], f32)
                RX2f = const.tile([K2, N], f32)
                RY2f = const.tile([K2, N], f32)
                nc.sync.dma_start(out=L2f, in_=L2_d[:])
                nc.sync.dma_start(out=RX2f, in_=RX2_d[:])
                nc.sync.dma_start(out=RY2f, in_=RY2_d[:])
                L2r = const.tile([K2, N], mmdt)
                RX2r = const.tile([K2, N], mmdt)
                RY2r = const.tile([K2, N], mmdt)
                nc.vector.tensor_copy(L2r, L2f)
                nc.vector.tensor_copy(RX2r, RX2f)
                nc.vector.tensor_copy(RY2r, RY2f)

            mins_x = outs.tile([P, n_tiles * NB], f32)
            mins_y = outs.tile([P, n_tiles * NB], f32)

            for _ in range(repeat):
                for bi in range(NB):
                    lhs = Lr[:, bi * P:(bi + 1) * P]
                    if bf16x2:
                        mats = ((RXr, RX2r, mins_x, True), (RYr, RY2r, mins_y, False))
                    else:
                        mats = ((RXr, None, mins_x, True), (RYr, None, mins_y, False))
                    for R, R2, mins, is_xx in mats:
                        for h in range(n_tiles):
                            p = psum.tile([P, red_w], f32, tag="p")
                            for c in range(n_ch):
                                col0 = h * red_w + c * mm_w
                                diag_here = is_xx and col0 <= bi * P < col0 + mm_w
                                sl = p[:, c * mm_w:(c + 1) * mm_w]
                                last = not (diag_here or bf16x2)
                                nc.tensor.matmul(
                                    sl, lhs, R[:, col0:col0 + mm_w],
                                    start=True, stop=last)
                                if bf16x2:
                                    nc.tensor.matmul(
                                        sl, L2r[:, bi * P:(bi + 1) * P],
                                        R2[:, col0:col0 + mm_w],
                                        start=False, stop=not diag_here,
                                        skip_group_check=True)
                                if diag_here:
                                    off = c * mm_w + (bi * P - col0)
                                    nc.tensor.matmul(
                                        p[:, off:off + P], EYEr[:], EYEr[:],
                                        start=False, stop=True,
                                        skip_group_check=True)
                            nc.vector.tensor_reduce(
                                out=mins[:, n_tiles * bi + h:n_tiles * bi + h + 1],
                                in_=p[:], axis=mybir.AxisListType.X,
                                op=mybir.AluOpType.min)

            nc.sync.dma_start(out=MX_d[:], in_=mins_x)
            nc.sync.dma_start(out=MY_d[:], in_=mins_y)

    nc.finalize()
    return nc


def _get_nc(repeat=1, mmdt_name=None, mm_w=None, red_w=None):
    key = (repeat, mmdt_name or MM_DTYPE, mm_w or MM_W, red_w or RED_W)
    if key not in _cache:
        _cache[key] = _build(repeat, mmdt_name, mm_w, red_w)
    return _cache[key]


def _hi_round(v):
    # hi part must be exactly representable in the matmul dtype
    if MM_DTYPE == "float16":
        return v.astype(np.float32).astype(np.float16).astype(np.float64)
    return v.astype(np.float32).astype(ml_dtypes.bfloat16).astype(np.float64)


def _bf16(v):
    return v.astype(np.float32).astype(ml_dtypes.bfloat16).astype(np.float64)


def _prep_maps(X, Y):
    X = np.asarray(X, dtype=np.float32)
    Y = np.asarray(Y, dtype=np.float32)
    eye = (np.eye(P) * SQRT_BIG).astype(np.float32)
    in_maps = []
    x2_all = []
    for b in range(B):
        Xb = X[b].astype(np.float64)
        Yb = Y[b].astype(np.float64)
        x2 = (Xb * Xb).sum(1)
        y2 = (Yb * Yb).sum(1)
        ones = np.ones((1, N), dtype=np.float64)
        if MM_DTYPE == "bf16x2":
            Xh = _bf16(Xb); Xl = Xb - Xh
            Yh = _bf16(Yb); Yl = Yb - Yh
            x2h = _bf16(x2); y2h = _bf16(y2)
            L = np.concatenate([-2.0 * Xh.T, ones, ones], 0).astype(np.float32)
            RX = np.concatenate([Xh.T, x2h[None], (x2 - x2h)[None]], 0).astype(np.float32)
            RY = np.concatenate([Yh.T, y2h[None], (y2 - y2h)[None]], 0).astype(np.float32)
            L2 = np.concatenate([-2.0 * Xh.T, -2.0 * Xl.T], 0).astype(np.float32)
            RX2 = np.concatenate([Xl.T, Xh.T], 0).astype(np.float32)
            RY2 = np.concatenate([Yl.T, Yh.T], 0).astype(np.float32)
            in_maps.append({"L": L, "RX": RX, "RY": RY,
                            "L2": L2, "RX2": RX2, "RY2": RY2, "EYE": eye})
        else:
            x2h = _hi_round(x2)
            y2h = _hi_round(y2)
            RX = np.concatenate([Xb.T, x2h[None], (x2 - x2h)[None]], 0).astype(np.float32)
            RY = np.concatenate([Yb.T, y2h[None], (y2 - y2h)[None]], 0).astype(np.float32)
            in_maps.append({"RX": RX, "RY": RY, "EYE": eye})
        x2_all.append(x2)
    return in_maps, x2_all


def _postprocess(results, x2_all):
    out = np.zeros(B, dtype=np.float64)
    for b in range(B):
        mx = results[b]["MX"].astype(np.float64)  # [P, n_tiles*NB]
        my = results[b]["MY"].astype(np.float64)
        nt = mx.shape[1] // NB
        # [p, bi, h] -> min over tiles -> [p, bi] -> row i = bi*P + p
        d2x = mx.reshape(P, NB, nt).min(2).T.reshape(-1)
        d2y = my.reshape(P, NB, nt).min(2).T.reshape(-1)
        d2x = d2x + x2_all[b]
        d2y = d2y + x2_all[b]
        d2x = np.maximum(d2x, EPS)
        d2y = np.maximum(d2y, EPS)
        out[b] = 0.5 * np.mean(np.log(d2x) - np.log(d2y))
    return out.astype(np.float32)


def _run_with_retry(nc, in_maps):
    for attempt in range(2):
        try:
            return run_bass_kernel_spmd(nc, in_maps, core_ids=list(range(B))).results
        except Exception:
            time.sleep(3)
    # last resort: one batch at a time, skipping wedged cores
    results = [None] * B
    for b in range(B):
        for c in range(8):
            core = (b + c) % 8
            try:
                results[b] = run_bass_kernel_spmd(
                    nc, [in_maps[b]], core_ids=[core]).results[0]
                break
            except Exception:
                continue
        if results[b] is None:
            raise RuntimeError("all cores failed")
    return results


def kernel(X, Y):
    nc = _get_nc(repeat=1)
    in_maps, x2_all = _prep_maps(X, Y)
    results = _run_with_retry(nc, in_maps)
    return _postprocess(results, x2_all)


if __name__ == "__main__":
    rng = np.random.default_rng(0)
    X = rng.standard_normal((B, N, D)).astype(np.float32)
    Y = rng.standard_normal((B, N, D)).astype(np.float32)
    print(kernel(X, Y))
